# revision 7
# baseline (speedup 1.0000x reference)
"""CSPN 3x3 propagation on 8 trn2 NeuronCores (batch-parallel), bf16.

out[y, x] = sum_{i,j} g[3i+j, y+1, x+1] * hn[y+1-i, x+1-j]
  (center tap i=j=1 uses h0; hn/h0 zero-padded outside [0,H)x[0,W))

All wire traffic is bf16 (host casts f32->bf16 inside kernel(); output
is stored bf16 and upcast on host — rel err stays well under the 2e-2
gate). ~11.7 MB per core vs 21.6 MB for the f32 version.

Per chunk of <=126 output rows:
- DVE computes the 9 tap products in 2x bf16 perf mode (~0.9us each;
  2x requires 4-byte-aligned element offsets on EVERY operand, and
  GpSimd must stay idle — its tensor_tensor poisons DVE to 0.25x).
  Products are emitted band-by-band (p_{3a}, p_{3a+1} -> pair-sum qa_a,
  then p_{3a+2}) so when the last guide band lands only ~3.5us of DVE
  work remains — short pipeline tail.
- PE shift-sums 6 streams per strip (qa_0,p2,qa_1,p5,qa_2,p8; shift
  d = 2-band via exact 0/1 matrices) into PSUM, stream-outer order so
  matmuls chase the data as it's produced.
- Scalar copies PSUM f32 -> SBUF bf16 (512/512/192 strips); stores are
  issued per-strip on the last chunk to overlap the drain.

DMA: guide is HOST-interleaved row-major [row, 9 planes, 1216] so a
3-plane band is 7296 B contiguous per row; max_dma_last_dim=4000 splits
that into 3648 B descriptors — the HWDGE ring dispatches ~1 descriptor
per ~19 ns, so bigger descriptors raise aggregate bandwidth (2432 B
gave ~250 GB/s; 3648 B targets ~300+). One dma_start per 64/112-row
half per band. hn/hh loads are 2436/2432 B descriptors.

Layouts (per core, B=1):
  guide -> [368, 10944] bf16: pad row, then row r = all 9 planes'
           row r-1 (orig cols 1..1217) concatenated, zero tail
  hn    -> [368, 1218] bf16: row r = hn[r-1] at cols 1..1216, else 0
  hh    -> [736, 1216] bf16: row pair r = (hn[r-1], h0[r-1])
  out   -> [352, 1216] bf16
"""

import numpy as np
import ml_dtypes

import concourse.bacc as bacc
import concourse.mybir as mybir
from concourse import tile
from concourse.ap import AP
from concourse.bass_utils import run_bass_kernel_spmd

BF16 = mybir.dt.bfloat16
F32 = mybir.dt.float32
MUL = mybir.AluOpType.mult
ADD = mybir.AluOpType.add

B, H, W = 8, 352, 1216
HP, WPAD = H + 2, W + 2        # 354, 1218
GW = 9 * W                     # 10944 guide row width
SROWS = 368
N_CORES = 8
CHUNKS = [(0, 126, 128), (126, 126, 128), (252, 100, 112)]  # (y0, R, KL)
STRIPS = [(0, 512), (512, 512), (1024, 192)]


def make_shift_mats():
    """S_d[k, m] = 1 iff k == m + d, d in {0,1,2}; packed [128, 378] bf16."""
    sm = np.zeros((128, 3 * 126), ml_dtypes.bfloat16)
    for d in range(3):
        for m in range(126):
            sm[m + d, d * 126 + m] = 1.0
    return sm


def prep_core_inputs(guide_b: np.ndarray, hn_b: np.ndarray, h0_b: np.ndarray,
                     sm: np.ndarray) -> dict:
    """guide_b [9, 354, 1218] f32, hn_b/h0_b [352, 1216] f32 -> bf16 dram dict."""
    gp = np.zeros((SROWS, GW), ml_dtypes.bfloat16)
    g = np.asarray(guide_b, np.float32)[:, :, 1:1 + W].astype(ml_dtypes.bfloat16)
    gp[1:1 + HP] = g.transpose(1, 0, 2).reshape(HP, GW)
    hnp = np.zeros((SROWS, WPAD), ml_dtypes.bfloat16)
    hnp[1:1 + H, 1:1 + W] = hn_b
    hh = np.zeros((SROWS, 2, W), ml_dtypes.bfloat16)
    hh[1:1 + H, 0, :] = hn_b
    hh[1:1 + H, 1, :] = h0_b
    return {"guide": gp, "hn": hnp, "hh": hh.reshape(2 * SROWS, W), "smat": sm}


def build():
    nc = bacc.Bacc(enable_partition_id=False)
    g_d = nc.dram_tensor("guide", [SROWS, GW], BF16, kind="ExternalInput")
    hn_d = nc.dram_tensor("hn", [SROWS, WPAD], BF16, kind="ExternalInput")
    hh_d = nc.dram_tensor("hh", [2 * SROWS, W], BF16, kind="ExternalInput")
    sm_d = nc.dram_tensor("smat", [128, 3 * 126], BF16, kind="ExternalInput")
    out_d = nc.dram_tensor("out", [H, W], BF16, kind="ExternalOutput")

    with tile.TileContext(nc) as tc:
        with tc.tile_pool(name="const", bufs=1) as cpool, \
             tc.tile_pool(name="gpool", bufs=3) as gpool, \
             tc.tile_pool(name="spool", bufs=3) as spool, \
             tc.tile_pool(name="ppool", bufs=2) as ppool, \
             tc.tile_pool(name="opool", bufs=2) as opool, \
             tc.tile_pool(name="psum", bufs=2, space="PSUM") as pspool:

            smt = cpool.tile([128, 3 * 126], BF16)
            nc.sync.dma_start(out=smt[0:64, :], in_=sm_d[0:64, :])
            nc.scalar.dma_start(out=smt[64:128, :], in_=sm_d[64:128, :])

            engs = [nc.sync, nc.scalar]

            def chunk_body(ci, y0, R, KL):
                hnt = spool.tile([128, WPAD], BF16, tag="hn", name="hnt")
                hht = spool.tile([128, 2, W], BF16, tag="hh", name="hht")
                gt = gpool.tile([128, 9, W], BF16, tag="g", name="gt")

                halves = [(0, 64), (64, 64)] if KL == 128 else [(0, 112)]
                for hi, (r0, rows) in enumerate(halves):
                    e = engs[(ci + hi) % 2]
                    e2 = engs[(ci + hi + 1) % 2]
                    e.dma_start(out=hnt[r0:r0 + rows, :],
                                in_=hn_d[y0 + r0:y0 + r0 + rows, :])
                    e2.dma_start(out=hht[r0:r0 + rows],
                                 in_=hh_d[2 * (y0 + r0):2 * (y0 + r0 + rows), :])
                    # guide band a: tile row k, slots 3a..3a+2 <- padded row
                    # y0+a+r0+k, cols [3a*W, 3a*W+3*W)
                    for a in range(3):
                        engs[(ci + hi + a) % 2].dma_start(
                            out=gt[r0:r0 + rows, 3 * a:3 * a + 3, :],
                            in_=AP(g_d, (y0 + a + r0) * GW + 3 * a * W,
                                   [[GW, rows], [1, 3 * W]]),
                            max_dma_last_dim=4000)

                def src_for(t):
                    i, j = t // 3, t % 3
                    if t == 4:
                        return hht[0:KL, 1, :]
                    if j == 1:
                        return hht[0:KL, 0, :]
                    return hnt[0:KL, 2 - j:2 - j + W]

                # DVE band-by-band: p3a, p3a+1 -> qa_a pair-sum; p3a+2 raw
                pt = ppool.tile([128, 9, W], BF16, tag="p", name="pt")
                qa = ppool.tile([128, 3, W], BF16, tag="q", name="qt")
                for a in range(3):
                    t0, t1, t2 = 3 * a, 3 * a + 1, 3 * a + 2
                    nc.vector.tensor_tensor(pt[0:KL, t0], gt[0:KL, t0], src_for(t0), MUL)
                    nc.vector.tensor_tensor(pt[0:KL, t1], gt[0:KL, t1], src_for(t1), MUL)
                    nc.vector.tensor_tensor(qa[0:KL, a], pt[0:KL, t0], pt[0:KL, t1], ADD)
                    nc.vector.tensor_tensor(pt[0:KL, t2], gt[0:KL, t2], src_for(t2), MUL)

                # PE: 6 streams x 3 strips, stream-outer so mms chase the data
                psts = [pspool.tile([126, 512], F32, tag=f"ps{s}", name=f"ps{s}")
                        for s in range(len(STRIPS))]
                streams = [(qa, 0, 2), (pt, 2, 2), (qa, 1, 1),
                           (pt, 5, 1), (qa, 2, 0), (pt, 8, 0)]
                for mi, (tl, slot, d) in enumerate(streams):
                    for s, (w0, N) in enumerate(STRIPS):
                        nc.tensor.matmul(psts[s][0:R, 0:N],
                                         smt[0:KL, d * 126:d * 126 + R],
                                         tl[0:KL, slot, w0:w0 + N],
                                         start=(mi == 0), stop=(mi == 5))

                # PSUM f32 -> SBUF bf16 -> HBM
                ot = opool.tile([128, W], BF16, tag="out", name="ot")
                if R == 126:
                    for s, (w0, N) in enumerate(STRIPS):
                        nc.scalar.copy(out=ot[0:R, w0:w0 + N], in_=psts[s][0:R, 0:N])
                    for si, (r0, rows) in enumerate([(0, 64), (64, R - 64)]):
                        engs[(ci + si) % 2].dma_start(
                            out=out_d[y0 + r0:y0 + r0 + rows, :],
                            in_=ot[r0:r0 + rows, :])
                else:
                    # last chunk: store each strip as soon as its copy lands
                    for s, (w0, N) in enumerate(STRIPS):
                        nc.scalar.copy(out=ot[0:R, w0:w0 + N], in_=psts[s][0:R, 0:N])
                        engs[(ci + s) % 2].dma_start(
                            out=out_d[y0:y0 + R, w0:w0 + N],
                            in_=ot[0:R, w0:w0 + N])

            for ci, (y0, R, KL) in enumerate(CHUNKS):
                chunk_body(ci, y0, R, KL)

    nc.finalize()
    return nc


_nc_cache = {}


def _get_nc():
    if "nc" not in _nc_cache:
        _nc_cache["nc"] = build()
    return _nc_cache["nc"]


def kernel(guide_weight: np.ndarray, hn: np.ndarray, h0: np.ndarray) -> np.ndarray:
    """Full inputs: guide_weight [8,9,354,1218], hn/h0 [8,1,352,1216] f32.
    Returns [8,1,352,1216] f32."""
    nc = _get_nc()
    sm = make_shift_mats()
    in_maps = [prep_core_inputs(guide_weight[b], hn[b, 0], h0[b, 0], sm)
               for b in range(B)]
    res = run_bass_kernel_spmd(nc, in_maps, list(range(N_CORES)))
    out = np.stack([np.asarray(res.results[b]["out"]) for b in range(B)], axis=0)
    return out[:, None].astype(np.float32)


# revision 8
# speedup vs baseline: 1.1291x; 1.1291x over previous
"""CSPN 3x3 propagation on 8 trn2 NeuronCores (batch-parallel), bf16.

out[y, x] = sum_{i,j} g[3i+j, y+1, x+1] * hn[y+1-i, x+1-j]
  (center tap i=j=1 uses h0; hn/h0 zero-padded outside [0,H)x[0,W))

All wire traffic is bf16 (host casts f32->bf16 inside kernel(); output
is stored bf16 and upcast on host — rel err stays well under the 2e-2
gate). ~11.7 MB per core vs 21.6 MB for the f32 version.

Per chunk of <=126 output rows:
- DVE computes the 9 tap products in 2x bf16 perf mode (~0.9us each;
  2x requires 4-byte-aligned element offsets on EVERY operand, and
  GpSimd must stay idle — its tensor_tensor poisons DVE to 0.25x).
  Bands 0/1 products come first and their pairs (p0+p1, p3+p4) are
  pre-summed with ONE strided 2-slot add; band 2's products are emitted
  LAST and streamed raw to the PE, so when the final guide band lands
  only ~3 products + 3 PE streams remain — short pipeline tail.
- PE shift-sums 7 streams per strip (q0,q1,p2,p5,p6,p7,p8; shift
  d = 2-band via exact 0/1 matrices) into PSUM, stream-outer order so
  matmuls chase the data as it's produced.
- Scalar copies PSUM f32 -> SBUF bf16 (512/512/192 strips); stores are
  issued per-strip on the last chunk to overlap the drain.

DMA: one dma_start per 64/112-row half covers a whole 3-plane guide
band via a 3D access pattern — the plane stride keeps descriptors at
2432 B, the sweet spot measured for the 16 SDMA engines. Guide tile for
tap t=3i+j is row-shifted (first row y0+i-1) so products are
partition-aligned to hn rows.

Layouts (per core, B=1):
  guide -> [3200, 1216] bf16: zero row, then 9 planes of [354, 1216]
           (orig cols 1..1217), zero tail
  hn    -> [368, 1218] bf16: row r = hn[r-1] at cols 1..1216, else 0
  hh    -> [736, 1216] bf16: row pair r = (hn[r-1], h0[r-1])
  out   -> [352, 1216] bf16
"""

import numpy as np
import ml_dtypes

import concourse.bacc as bacc
import concourse.mybir as mybir
from concourse import tile
from concourse.ap import AP
from concourse.bass_utils import run_bass_kernel_spmd

BF16 = mybir.dt.bfloat16
F32 = mybir.dt.float32
MUL = mybir.AluOpType.mult
ADD = mybir.AluOpType.add

B, H, W = 8, 352, 1216
HP, WPAD = H + 2, W + 2        # 354, 1218
GROWS = 1 + 9 * HP + 13        # 3200
SROWS = 368
N_CORES = 8
CHUNKS = [(0, 126, 128), (126, 126, 128), (252, 100, 112)]  # (y0, R, KL)
STRIPS = [(0, 512), (512, 512), (1024, 192)]


def make_shift_mats():
    """S_d[k, m] = 1 iff k == m + d, d in {0,1,2}; packed [128, 378] bf16."""
    sm = np.zeros((128, 3 * 126), ml_dtypes.bfloat16)
    for d in range(3):
        for m in range(126):
            sm[m + d, d * 126 + m] = 1.0
    return sm


def prep_core_inputs(guide_b: np.ndarray, hn_b: np.ndarray, h0_b: np.ndarray,
                     sm: np.ndarray) -> dict:
    """guide_b [9, 354, 1218] f32, hn_b/h0_b [352, 1216] f32 -> bf16 dram dict."""
    gp = np.zeros((GROWS, W), ml_dtypes.bfloat16)
    gp[1:1 + 9 * HP] = np.asarray(guide_b, np.float32)[:, :, 1:1 + W].reshape(9 * HP, W)
    hnp = np.zeros((SROWS, WPAD), ml_dtypes.bfloat16)
    hnp[1:1 + H, 1:1 + W] = hn_b
    hh = np.zeros((SROWS, 2, W), ml_dtypes.bfloat16)
    hh[1:1 + H, 0, :] = hn_b
    hh[1:1 + H, 1, :] = h0_b
    return {"guide": gp, "hn": hnp, "hh": hh.reshape(2 * SROWS, W), "smat": sm}


def build():
    nc = bacc.Bacc(enable_partition_id=False)
    g_d = nc.dram_tensor("guide", [GROWS, W], BF16, kind="ExternalInput")
    hn_d = nc.dram_tensor("hn", [SROWS, WPAD], BF16, kind="ExternalInput")
    hh_d = nc.dram_tensor("hh", [2 * SROWS, W], BF16, kind="ExternalInput")
    sm_d = nc.dram_tensor("smat", [128, 3 * 126], BF16, kind="ExternalInput")
    out_d = nc.dram_tensor("out", [H, W], BF16, kind="ExternalOutput")

    with tile.TileContext(nc) as tc:
        with tc.tile_pool(name="const", bufs=1) as cpool, \
             tc.tile_pool(name="gpool", bufs=3) as gpool, \
             tc.tile_pool(name="spool", bufs=3) as spool, \
             tc.tile_pool(name="ppool", bufs=2) as ppool, \
             tc.tile_pool(name="opool", bufs=2) as opool, \
             tc.tile_pool(name="psum", bufs=2, space="PSUM") as pspool:

            smt = cpool.tile([128, 3 * 126], BF16)
            nc.sync.dma_start(out=smt[0:64, :], in_=sm_d[0:64, :])
            nc.scalar.dma_start(out=smt[64:128, :], in_=sm_d[64:128, :])

            engs = [nc.sync, nc.scalar]

            def chunk_body(ci, y0, R, KL):
                hnt = spool.tile([128, WPAD], BF16, tag="hn", name="hnt")
                hht = spool.tile([128, 2, W], BF16, tag="hh", name="hht")
                gt = gpool.tile([128, 9, W], BF16, tag="g", name="gt")

                halves = [(0, 64), (64, 64)] if KL == 128 else [(0, 112)]
                for hi, (r0, rows) in enumerate(halves):
                    e = engs[(ci + hi) % 2]
                    e2 = engs[(ci + hi + 1) % 2]
                    e.dma_start(out=hnt[r0:r0 + rows, :],
                                in_=hn_d[y0 + r0:y0 + r0 + rows, :])
                    e2.dma_start(out=hht[r0:r0 + rows],
                                 in_=hh_d[2 * (y0 + r0):2 * (y0 + r0 + rows), :])
                    # guide band a: planes 3a..3a+2, tile row k <- flat row
                    # 1 + (3a+p)*HP + y0 + a - 1 + k
                    for a in range(3):
                        base = 1 + 3 * a * HP + y0 + a - 1 + r0
                        engs[(ci + hi + a) % 2].dma_start(
                            out=gt[r0:r0 + rows, 3 * a:3 * a + 3, :],
                            in_=AP(g_d, base * W,
                                   [[W, rows], [HP * W, 3], [1, W]]))

                def src_for(t):
                    i, j = t // 3, t % 3
                    if t == 4:
                        return hht[0:KL, 1, :]
                    if j == 1:
                        return hht[0:KL, 0, :]
                    return hnt[0:KL, 2 - j:2 - j + W]

                # DVE: bands 0/1 first with ONE strided pair-sum, band 2 last
                pt = ppool.tile([128, 9, W], BF16, tag="p", name="pt")
                q = ppool.tile([128, 2, W], BF16, tag="q", name="qt")
                for t in (0, 1, 3, 4):
                    nc.vector.tensor_tensor(pt[0:KL, t], gt[0:KL, t], src_for(t), MUL)
                nc.vector.tensor_tensor(
                    q[0:KL],
                    AP(pt.tensor, 0, [[9 * W, KL], [3 * W, 2], [1, W]]),
                    AP(pt.tensor, W, [[9 * W, KL], [3 * W, 2], [1, W]]),
                    ADD)
                for t in (2, 5, 6, 7, 8):
                    nc.vector.tensor_tensor(pt[0:KL, t], gt[0:KL, t], src_for(t), MUL)

                # PE: 7 streams x 3 strips, stream-outer so mms chase the data
                psts = [pspool.tile([126, 512], F32, tag=f"ps{s}", name=f"ps{s}")
                        for s in range(len(STRIPS))]
                streams = [(q, 0, 2), (q, 1, 1), (pt, 2, 2), (pt, 5, 1),
                           (pt, 6, 0), (pt, 7, 0), (pt, 8, 0)]
                for mi, (tl, slot, d) in enumerate(streams):
                    for s, (w0, N) in enumerate(STRIPS):
                        nc.tensor.matmul(psts[s][0:R, 0:N],
                                         smt[0:KL, d * 126:d * 126 + R],
                                         tl[0:KL, slot, w0:w0 + N],
                                         start=(mi == 0), stop=(mi == 6))

                # PSUM f32 -> SBUF bf16 -> HBM
                ot = opool.tile([128, W], BF16, tag="out", name="ot")
                if R == 126:
                    for s, (w0, N) in enumerate(STRIPS):
                        nc.scalar.copy(out=ot[0:R, w0:w0 + N], in_=psts[s][0:R, 0:N])
                    for si, (r0, rows) in enumerate([(0, 64), (64, R - 64)]):
                        engs[(ci + si) % 2].dma_start(
                            out=out_d[y0 + r0:y0 + r0 + rows, :],
                            in_=ot[r0:r0 + rows, :])
                else:
                    # last chunk: store each strip as soon as its copy lands
                    for s, (w0, N) in enumerate(STRIPS):
                        nc.scalar.copy(out=ot[0:R, w0:w0 + N], in_=psts[s][0:R, 0:N])
                        engs[(ci + s) % 2].dma_start(
                            out=out_d[y0:y0 + R, w0:w0 + N],
                            in_=ot[0:R, w0:w0 + N])

            for ci, (y0, R, KL) in enumerate(CHUNKS):
                chunk_body(ci, y0, R, KL)

    nc.finalize()
    return nc


_nc_cache = {}


def _get_nc():
    if "nc" not in _nc_cache:
        _nc_cache["nc"] = build()
    return _nc_cache["nc"]


def kernel(guide_weight: np.ndarray, hn: np.ndarray, h0: np.ndarray) -> np.ndarray:
    """Full inputs: guide_weight [8,9,354,1218], hn/h0 [8,1,352,1216] f32.
    Returns [8,1,352,1216] f32."""
    nc = _get_nc()
    sm = make_shift_mats()
    in_maps = [prep_core_inputs(guide_weight[b], hn[b, 0], h0[b, 0], sm)
               for b in range(B)]
    res = run_bass_kernel_spmd(nc, in_maps, list(range(N_CORES)))
    out = np.stack([np.asarray(res.results[b]["out"]) for b in range(B)], axis=0)
    return out[:, None].astype(np.float32)


# revision 10
# speedup vs baseline: 1.2240x; 1.0840x over previous
"""CSPN 3x3 propagation on 8 trn2 NeuronCores (batch-parallel), bf16.

out[y, x] = sum_{i,j} g[3i+j, y+1, x+1] * hn[y+1-i, x+1-j]
  (center tap i=j=1 uses h0; hn/h0 zero-padded outside [0,H)x[0,W))

All wire traffic is bf16 (host casts f32->bf16 inside kernel(); output
is stored bf16 and upcast on host — rel err stays well under the 2e-2
gate). ~11.7 MB per core vs 21.6 MB for the f32 version.

Per chunk of <=126 output rows:
- DVE computes the 9 tap products in 2x bf16 perf mode (~0.9us each;
  2x requires 4-byte-aligned element offsets on EVERY operand, and
  GpSimd must stay idle — its tensor_tensor poisons DVE to 0.25x).
  Bands 0/1 products come first and their pairs (p0+p1, p3+p4) are
  pre-summed with ONE strided 2-slot add; band 2's products are emitted
  LAST and streamed raw to the PE, so when the final guide band lands
  only ~3 products + 3 PE streams remain — short pipeline tail.
- PE shift-sums 7 streams per strip (q0,q1,p2,p5,p6,p7,p8; shift
  d = 2-band via exact 0/1 matrices) into PSUM, stream-outer order so
  matmuls chase the data as it's produced.
- Scalar copies PSUM f32 -> SBUF bf16 (512/512/192 strips); stores are
  issued per-strip on the last chunk to overlap the drain.

DMA: one dma_start per 64/112-row half covers a whole 3-plane guide
band via a 3D access pattern — the plane stride keeps descriptors at
2432 B, the sweet spot measured for the 16 SDMA engines. Guide tile for
tap t=3i+j is row-shifted (first row y0+i-1) so products are
partition-aligned to hn rows.

Layouts (per core, B=1):
  guide -> [3200, 1216] bf16: zero row, then 9 planes of [354, 1216]
           (orig cols 1..1217), zero tail
  hn    -> [368, 1218] bf16: row r = hn[r-1] at cols 1..1216, else 0
  hh    -> [736, 1216] bf16: row pair r = (hn[r-1], h0[r-1])
  out   -> [352, 1216] bf16
"""

import numpy as np
import ml_dtypes

import concourse.bacc as bacc
import concourse.mybir as mybir
from concourse import tile
from concourse.ap import AP
from concourse.bass_utils import run_bass_kernel_spmd

BF16 = mybir.dt.bfloat16
F32 = mybir.dt.float32
MUL = mybir.AluOpType.mult
ADD = mybir.AluOpType.add

B, H, W = 8, 352, 1216
HP, WPAD = H + 2, W + 2        # 354, 1218
GROWS = 1 + 9 * HP + 13        # 3200
SROWS = 368
N_CORES = 8
CHUNKS = [(0, 126, 128), (126, 126, 128), (252, 100, 112)]  # (y0, R, KL)
STRIPS = [(0, 512), (512, 512), (1024, 192)]


def make_shift_mats():
    """S_d[k, m] = 1 iff k == m + d, d in {0,1,2}; packed [128, 378] bf16."""
    sm = np.zeros((128, 3 * 126), ml_dtypes.bfloat16)
    for d in range(3):
        for m in range(126):
            sm[m + d, d * 126 + m] = 1.0
    return sm


def prep_core_inputs(guide_b: np.ndarray, hn_b: np.ndarray, h0_b: np.ndarray,
                     sm: np.ndarray) -> dict:
    """guide_b [9, 354, 1218] f32, hn_b/h0_b [352, 1216] f32 -> bf16 dram dict."""
    gp = np.zeros((GROWS, W), ml_dtypes.bfloat16)
    gp[1:1 + 9 * HP] = np.asarray(guide_b, np.float32)[:, :, 1:1 + W].reshape(9 * HP, W)
    hnp = np.zeros((SROWS, WPAD), ml_dtypes.bfloat16)
    hnp[1:1 + H, 1:1 + W] = hn_b
    hh = np.zeros((SROWS, 2, W), ml_dtypes.bfloat16)
    hh[1:1 + H, 0, :] = hn_b
    hh[1:1 + H, 1, :] = h0_b
    return {"guide": gp, "hn": hnp, "hh": hh.reshape(2 * SROWS, W), "smat": sm}


def build():
    nc = bacc.Bacc(enable_partition_id=False)
    g_d = nc.dram_tensor("guide", [GROWS, W], BF16, kind="ExternalInput")
    hn_d = nc.dram_tensor("hn", [SROWS, WPAD], BF16, kind="ExternalInput")
    hh_d = nc.dram_tensor("hh", [2 * SROWS, W], BF16, kind="ExternalInput")
    sm_d = nc.dram_tensor("smat", [128, 3 * 126], BF16, kind="ExternalInput")
    out_d = nc.dram_tensor("out", [H, W], BF16, kind="ExternalOutput")

    with tile.TileContext(nc) as tc:
        with tc.tile_pool(name="const", bufs=1) as cpool, \
             tc.tile_pool(name="gpool", bufs=3) as gpool, \
             tc.tile_pool(name="spool", bufs=3) as spool, \
             tc.tile_pool(name="ppool", bufs=2) as ppool, \
             tc.tile_pool(name="opool", bufs=2) as opool, \
             tc.tile_pool(name="psum", bufs=2, space="PSUM") as pspool:

            smt = cpool.tile([128, 3 * 126], BF16)
            nc.sync.dma_start(out=smt[0:64, :], in_=sm_d[0:64, :])
            nc.scalar.dma_start(out=smt[64:128, :], in_=sm_d[64:128, :])

            engs = [nc.sync, nc.scalar]
            smalls = {}

            def issue_smalls(ci):
                """Issue chunk ci's hn/hh loads (one chunk of lookahead)."""
                y0, R, KL = CHUNKS[ci]
                hnt = spool.tile([128, WPAD], BF16, tag="hn", name="hnt")
                hht = spool.tile([128, 2, W], BF16, tag="hh", name="hht")
                halves = [(0, 64), (64, 64)] if KL == 128 else [(0, 112)]
                for hi, (r0, rows) in enumerate(halves):
                    e = engs[(ci + hi) % 2]
                    e2 = engs[(ci + hi + 1) % 2]
                    e.dma_start(out=hnt[r0:r0 + rows, :],
                                in_=hn_d[y0 + r0:y0 + r0 + rows, :])
                    # hh slots loaded separately: strided DRAM rows keep the
                    # descriptors at 2432 B (one 4864 B load measured slower)
                    for sl in range(2):
                        e2.dma_start(
                            out=hht[r0:r0 + rows, sl, :],
                            in_=AP(hh_d, (2 * (y0 + r0) + sl) * W,
                                   [[2 * W, rows], [1, W]]))
                smalls[ci] = (hnt, hht)

            def chunk_body(ci, y0, R, KL):
                hnt, hht = smalls[ci]
                gt = gpool.tile([128, 9, W], BF16, tag="g", name="gt")

                halves = [(0, 64), (64, 64)] if KL == 128 else [(0, 112)]
                for hi, (r0, rows) in enumerate(halves):
                    # guide band a: planes 3a..3a+2, tile row k <- flat row
                    # 1 + (3a+p)*HP + y0 + a - 1 + k
                    for a in range(3):
                        base = 1 + 3 * a * HP + y0 + a - 1 + r0
                        engs[(ci + hi + a) % 2].dma_start(
                            out=gt[r0:r0 + rows, 3 * a:3 * a + 3, :],
                            in_=AP(g_d, base * W,
                                   [[W, rows], [HP * W, 3], [1, W]]))

                def src_for(t):
                    i, j = t // 3, t % 3
                    if t == 4:
                        return hht[0:KL, 1, :]
                    if j == 1:
                        return hht[0:KL, 0, :]
                    return hnt[0:KL, 2 - j:2 - j + W]

                # DVE: bands 0/1 first (pairs pre-summed with ONE strided
                # 2-slot add), band 2 last with its own pair-sum
                pt = ppool.tile([128, 9, W], BF16, tag="p", name="pt")
                q = ppool.tile([128, 2, W], BF16, tag="q", name="qt")
                qb = ppool.tile([128, W], BF16, tag="qb", name="qbt")
                for t in (0, 1, 3, 4):
                    nc.vector.tensor_tensor(pt[0:KL, t], gt[0:KL, t], src_for(t), MUL)
                nc.vector.tensor_tensor(
                    q[0:KL],
                    AP(pt.tensor, 0, [[9 * W, KL], [3 * W, 2], [1, W]]),
                    AP(pt.tensor, W, [[9 * W, KL], [3 * W, 2], [1, W]]),
                    ADD)
                for t in (2, 5, 6, 7):
                    nc.vector.tensor_tensor(pt[0:KL, t], gt[0:KL, t], src_for(t), MUL)
                nc.vector.tensor_tensor(qb[0:KL], pt[0:KL, 6], pt[0:KL, 7], ADD)
                nc.vector.tensor_tensor(pt[0:KL, 8], gt[0:KL, 8], src_for(8), MUL)

                # PE: 6 streams x 3 strips, stream-outer so mms chase the data
                psts = [pspool.tile([126, 512], F32, tag=f"ps{s}", name=f"ps{s}")
                        for s in range(len(STRIPS))]
                streams = [(q[0:KL, 0, :], 2), (q[0:KL, 1, :], 1),
                           (pt[0:KL, 2, :], 2), (pt[0:KL, 5, :], 1),
                           (qb[0:KL, :], 0), (pt[0:KL, 8, :], 0)]
                for mi, (mv, d) in enumerate(streams):
                    for s, (w0, N) in enumerate(STRIPS):
                        nc.tensor.matmul(psts[s][0:R, 0:N],
                                         smt[0:KL, d * 126:d * 126 + R],
                                         mv[:, w0:w0 + N],
                                         start=(mi == 0), stop=(mi == 5))

                # PSUM f32 -> SBUF bf16 -> HBM
                ot = opool.tile([128, W], BF16, tag="out", name="ot")
                if R == 126:
                    for s, (w0, N) in enumerate(STRIPS):
                        nc.scalar.copy(out=ot[0:R, w0:w0 + N], in_=psts[s][0:R, 0:N])
                    for si, (r0, rows) in enumerate([(0, 64), (64, R - 64)]):
                        engs[(ci + si) % 2].dma_start(
                            out=out_d[y0 + r0:y0 + r0 + rows, :],
                            in_=ot[r0:r0 + rows, :])
                else:
                    # last chunk: store each strip as soon as its copy lands
                    for s, (w0, N) in enumerate(STRIPS):
                        nc.scalar.copy(out=ot[0:R, w0:w0 + N], in_=psts[s][0:R, 0:N])
                        engs[(ci + s) % 2].dma_start(
                            out=out_d[y0:y0 + R, w0:w0 + N],
                            in_=ot[0:R, w0:w0 + N])

            issue_smalls(0)
            for ci, (y0, R, KL) in enumerate(CHUNKS):
                if ci + 1 < len(CHUNKS):
                    issue_smalls(ci + 1)
                chunk_body(ci, y0, R, KL)

    nc.finalize()
    return nc


_nc_cache = {}


def _get_nc():
    if "nc" not in _nc_cache:
        _nc_cache["nc"] = build()
    return _nc_cache["nc"]


def kernel(guide_weight: np.ndarray, hn: np.ndarray, h0: np.ndarray) -> np.ndarray:
    """Full inputs: guide_weight [8,9,354,1218], hn/h0 [8,1,352,1216] f32.
    Returns [8,1,352,1216] f32."""
    nc = _get_nc()
    sm = make_shift_mats()
    in_maps = [prep_core_inputs(guide_weight[b], hn[b, 0], h0[b, 0], sm)
               for b in range(B)]
    res = run_bass_kernel_spmd(nc, in_maps, list(range(N_CORES)))
    out = np.stack([np.asarray(res.results[b]["out"]) for b in range(B)], axis=0)
    return out[:, None].astype(np.float32)


# revision 13
# speedup vs baseline: 1.2528x; 1.0235x over previous
"""CSPN 3x3 propagation on 8 trn2 NeuronCores (batch-parallel), bf16.

out[y, x] = sum_{i,j} g[3i+j, y+1, x+1] * hn[y+1-i, x+1-j]
  (center tap i=j=1 uses h0; hn/h0 zero-padded outside [0,H)x[0,W))

All wire traffic is bf16 (host casts f32->bf16 inside kernel(); output
is stored bf16 and upcast on host — rel err stays well under the 2e-2
gate). ~11.7 MB per core vs 21.6 MB for the f32 version.

Per chunk of <=126 output rows:
- DVE computes the 9 tap products in 2x bf16 perf mode (~0.9us each;
  2x requires 4-byte-aligned element offsets on EVERY operand, and
  GpSimd must stay idle — its tensor_tensor poisons DVE to 0.25x).
  Bands 0/1 products come first and their pairs (p0+p1, p3+p4) are
  pre-summed with ONE strided 2-slot add; band 2's products are emitted
  LAST and streamed raw to the PE, so when the final guide band lands
  only ~3 products + 3 PE streams remain — short pipeline tail.
- PE shift-sums 7 streams per strip (q0,q1,p2,p5,p6,p7,p8; shift
  d = 2-band via exact 0/1 matrices) into PSUM, stream-outer order so
  matmuls chase the data as it's produced.
- Scalar copies PSUM f32 -> SBUF bf16 (512/512/192 strips); stores are
  issued per-strip on the last chunk to overlap the drain.

DMA: one dma_start per 64/112-row half covers a whole 3-plane guide
band via a 3D access pattern — the plane stride keeps descriptors at
2432 B, the sweet spot measured for the 16 SDMA engines. Guide tile for
tap t=3i+j is row-shifted (first row y0+i-1) so products are
partition-aligned to hn rows.

Layouts (per core, B=1):
  guide -> [3200, 1216] bf16: zero row, then 9 planes of [354, 1216]
           (orig cols 1..1217), zero tail
  hn    -> [368, 1218] bf16: row r = hn[r-1] at cols 1..1216, else 0
  hh    -> [736, 1216] bf16: row pair r = (hn[r-1], h0[r-1])
  out   -> [352, 1216] bf16
"""

import numpy as np
import ml_dtypes

import concourse.bacc as bacc
import concourse.mybir as mybir
from concourse import tile
from concourse.ap import AP
from concourse.bass_utils import run_bass_kernel_spmd

BF16 = mybir.dt.bfloat16
F32 = mybir.dt.float32
MUL = mybir.AluOpType.mult
ADD = mybir.AluOpType.add

B, H, W = 8, 352, 1216
HP, WPAD = H + 2, W + 2        # 354, 1218
GROWS = 1 + 9 * HP + 13        # 3200
SROWS = 368
N_CORES = 8
CHUNKS = [(0, 126, 128), (126, 126, 128), (252, 100, 112)]  # (y0, R, KL)
STRIPS = [(0, 512), (512, 512), (1024, 192)]


def make_shift_mats():
    """S_d[k, m] = 1 iff k == m + d, d in {0,1,2}; packed [128, 378] bf16."""
    sm = np.zeros((128, 3 * 126), ml_dtypes.bfloat16)
    for d in range(3):
        for m in range(126):
            sm[m + d, d * 126 + m] = 1.0
    return sm


def prep_core_inputs(guide_b: np.ndarray, hn_b: np.ndarray, h0_b: np.ndarray,
                     sm: np.ndarray) -> dict:
    """guide_b [9, 354, 1218] f32, hn_b/h0_b [352, 1216] f32 -> bf16 dram dict."""
    gp = np.zeros((GROWS, W), ml_dtypes.bfloat16)
    gp[1:1 + 9 * HP] = np.asarray(guide_b, np.float32)[:, :, 1:1 + W].reshape(9 * HP, W)
    hnp = np.zeros((SROWS, WPAD), ml_dtypes.bfloat16)
    hnp[1:1 + H, 1:1 + W] = hn_b
    hh = np.zeros((SROWS, 2, W), ml_dtypes.bfloat16)
    hh[1:1 + H, 0, :] = hn_b
    hh[1:1 + H, 1, :] = h0_b
    return {"guide": gp, "hn": hnp, "hh": hh.reshape(2 * SROWS, W), "smat": sm}


def build():
    nc = bacc.Bacc(enable_partition_id=False)
    g_d = nc.dram_tensor("guide", [GROWS, W], BF16, kind="ExternalInput")
    hn_d = nc.dram_tensor("hn", [SROWS, WPAD], BF16, kind="ExternalInput")
    hh_d = nc.dram_tensor("hh", [2 * SROWS, W], BF16, kind="ExternalInput")
    sm_d = nc.dram_tensor("smat", [128, 3 * 126], BF16, kind="ExternalInput")
    out_d = nc.dram_tensor("out", [H, W], BF16, kind="ExternalOutput")

    with tile.TileContext(nc) as tc:
        with tc.tile_pool(name="const", bufs=1) as cpool, \
             tc.tile_pool(name="gpool", bufs=3) as gpool, \
             tc.tile_pool(name="spool", bufs=3) as spool, \
             tc.tile_pool(name="ppool", bufs=2) as ppool, \
             tc.tile_pool(name="opool", bufs=2) as opool, \
             tc.tile_pool(name="psum", bufs=2, space="PSUM") as pspool:

            smt = cpool.tile([128, 3 * 126], BF16)
            nc.sync.dma_start(out=smt[0:64, :], in_=sm_d[0:64, :])
            nc.scalar.dma_start(out=smt[64:128, :], in_=sm_d[64:128, :])

            engs = [nc.sync, nc.scalar]
            smalls = {}

            def issue_smalls(ci):
                """Issue chunk ci's hn/hh loads (one chunk of lookahead)."""
                y0, R, KL = CHUNKS[ci]
                hnt = spool.tile([128, WPAD], BF16, tag="hn", name="hnt")
                hht = spool.tile([128, 2, W], BF16, tag="hh", name="hht")
                halves = [(0, 64), (64, 64)] if KL == 128 else [(0, 112)]
                for hi, (r0, rows) in enumerate(halves):
                    e = engs[(ci + hi) % 2]
                    e2 = engs[(ci + hi + 1) % 2]
                    e.dma_start(out=hnt[r0:r0 + rows, :],
                                in_=hn_d[y0 + r0:y0 + r0 + rows, :])
                    # hh slots loaded separately: strided DRAM rows keep the
                    # descriptors at 2432 B (one 4864 B load measured slower)
                    for sl in range(2):
                        e2.dma_start(
                            out=hht[r0:r0 + rows, sl, :],
                            in_=AP(hh_d, (2 * (y0 + r0) + sl) * W,
                                   [[2 * W, rows], [1, W]]))
                smalls[ci] = (hnt, hht)

            def chunk_body(ci, y0, R, KL):
                hnt, hht = smalls[ci]
                gt = gpool.tile([128, 9, W], BF16, tag="g", name="gt")

                # guide band a: planes 3a..3a+2, tile row k <- flat row
                # 1 + (3a+p)*HP + y0 + a - 1 + k. One full-KL transfer per
                # band (3D APs spray fine even at 128 rows), one per ring —
                # gpsimd SWDGE acts as a third ring at ~110 B/ns.
                rings = [nc.sync, nc.scalar, nc.gpsimd]
                for a in range(3):
                    base = 1 + 3 * a * HP + y0 + a - 1
                    rings[(ci + a) % 3].dma_start(
                        out=gt[0:KL, 3 * a:3 * a + 3, :],
                        in_=AP(g_d, base * W,
                               [[W, KL], [HP * W, 3], [1, W]]))

                def src_for(t):
                    i, j = t // 3, t % 3
                    if t == 4:
                        return hht[0:KL, 1, :]
                    if j == 1:
                        return hht[0:KL, 0, :]
                    return hnt[0:KL, 2 - j:2 - j + W]

                # DVE: bands 0/1 first (pairs pre-summed with ONE strided
                # 2-slot add), band 2 last with its own pair-sum
                pt = ppool.tile([128, 9, W], BF16, tag="p", name="pt")
                q = ppool.tile([128, 2, W], BF16, tag="q", name="qt")
                qb = ppool.tile([128, W], BF16, tag="qb", name="qbt")
                for t in (0, 1, 2, 3, 4, 5):
                    nc.vector.tensor_tensor(pt[0:KL, t], gt[0:KL, t], src_for(t), MUL)
                nc.vector.tensor_tensor(
                    q[0:KL],
                    AP(pt.tensor, 0, [[9 * W, KL], [3 * W, 2], [1, W]]),
                    AP(pt.tensor, W, [[9 * W, KL], [3 * W, 2], [1, W]]),
                    ADD)
                for t in (6, 7):
                    nc.vector.tensor_tensor(pt[0:KL, t], gt[0:KL, t], src_for(t), MUL)
                nc.vector.tensor_tensor(qb[0:KL], pt[0:KL, 6], pt[0:KL, 7], ADD)
                nc.vector.tensor_tensor(pt[0:KL, 8], gt[0:KL, 8], src_for(8), MUL)

                # PE: 6 streams x 3 strips, stream-outer so mms chase the data
                psts = [pspool.tile([126, 512], F32, tag=f"ps{s}", name=f"ps{s}")
                        for s in range(len(STRIPS))]
                streams = [(q[0:KL, 0, :], 2), (q[0:KL, 1, :], 1),
                           (pt[0:KL, 2, :], 2), (pt[0:KL, 5, :], 1),
                           (qb[0:KL, :], 0), (pt[0:KL, 8, :], 0)]
                for mi, (mv, d) in enumerate(streams):
                    for s, (w0, N) in enumerate(STRIPS):
                        nc.tensor.matmul(psts[s][0:R, 0:N],
                                         smt[0:KL, d * 126:d * 126 + R],
                                         mv[:, w0:w0 + N],
                                         start=(mi == 0), stop=(mi == 5))

                # PSUM f32 -> SBUF bf16 -> HBM (stores ride the gpsimd ring)
                ot = opool.tile([128, W], BF16, tag="out", name="ot")
                if R == 126:
                    for s, (w0, N) in enumerate(STRIPS):
                        nc.scalar.copy(out=ot[0:R, w0:w0 + N], in_=psts[s][0:R, 0:N])
                    nc.gpsimd.dma_start(out=out_d[y0:y0 + R, :], in_=ot[0:R, :])
                else:
                    # last chunk: store each strip as soon as its copy lands
                    for s, (w0, N) in enumerate(STRIPS):
                        nc.scalar.copy(out=ot[0:R, w0:w0 + N], in_=psts[s][0:R, 0:N])
                        [nc.gpsimd, nc.sync, nc.scalar][s].dma_start(
                            out=out_d[y0:y0 + R, w0:w0 + N],
                            in_=ot[0:R, w0:w0 + N])

            issue_smalls(0)
            for ci, (y0, R, KL) in enumerate(CHUNKS):
                if ci + 1 < len(CHUNKS):
                    issue_smalls(ci + 1)
                chunk_body(ci, y0, R, KL)

    nc.finalize()
    return nc


_nc_cache = {}


def _get_nc():
    if "nc" not in _nc_cache:
        _nc_cache["nc"] = build()
    return _nc_cache["nc"]


def kernel(guide_weight: np.ndarray, hn: np.ndarray, h0: np.ndarray) -> np.ndarray:
    """Full inputs: guide_weight [8,9,354,1218], hn/h0 [8,1,352,1216] f32.
    Returns [8,1,352,1216] f32."""
    nc = _get_nc()
    sm = make_shift_mats()
    in_maps = [prep_core_inputs(guide_weight[b], hn[b, 0], h0[b, 0], sm)
               for b in range(B)]
    res = run_bass_kernel_spmd(nc, in_maps, list(range(N_CORES)))
    out = np.stack([np.asarray(res.results[b]["out"]) for b in range(B)], axis=0)
    return out[:, None].astype(np.float32)


# revision 14
# speedup vs baseline: 1.3553x; 1.0818x over previous
"""CSPN 3x3 propagation on 8 trn2 NeuronCores (batch-parallel), bf16.

out[y, x] = sum_{i,j} g[3i+j, y+1, x+1] * hn[y+1-i, x+1-j]
  (center tap i=j=1 uses h0; hn/h0 zero-padded outside [0,H)x[0,W))

All wire traffic is bf16 (host casts f32->bf16 inside kernel(); output
is stored bf16 and upcast on host — rel err ~6e-3 vs the 2e-2 gate).
~10.8 MB per core vs 21.6 MB for the f32 version.

Compute, per chunk of <=126 output rows:
- DVE computes the 9 tap products in 2x bf16 perf mode (~0.8us each;
  2x requires 4-byte-aligned element offsets on EVERY operand, and
  GpSimd compute must stay off — it poisons DVE to 0.25x). The j=1
  taps (t=1, t=7) read hn at an odd offset and run 1x; that is cheaper
  than loading a second copy of hn. Bands 0/1 products come first
  (pairs pre-summed via ONE strided 2-slot add), band 2 last with its
  own pair-sum, so little DVE work remains after the last band lands.
- PE shift-sums 6 streams per strip (q0,q1,p2,p5,qb,p8; shift d=2-band
  via exact 0/1 matrices) into PSUM f32.
- Scalar copies PSUM -> SBUF bf16 (512/512/192 strips).

DMA schedule: three rings (sync HWDGE, scalar HWDGE, gpsimd SWDGE —
measured ~110 B/ns each, all spraying across the 16 SDMA engines).
Transfers are issued in GLOBAL just-in-time order: chunk c+1's first
bands are enqueued before chunk c's last band, so each ring's FIFO
delivers every band right when the DVE needs it and the final chunk's
early bands arrive well before the load stream ends. One full-KL
transfer per band (3D APs spray fine even at 128 rows, 2432 B
descriptors). Stores are emitted late so they never stall a ring.

Layouts (per core, B=1):
  guide -> [3200, 1216] bf16: zero row, then 9 planes of [354, 1216]
           (orig cols 1..1217), zero tail
  hn    -> [368, 1218] bf16: row r = hn[r-1] at cols 1..1216, else 0
  h0    -> [368, 1216] bf16: row r = h0[r-1]
  out   -> [352, 1216] bf16
"""

import numpy as np
import ml_dtypes

import concourse.bacc as bacc
import concourse.mybir as mybir
from concourse import tile
from concourse.ap import AP
from concourse.bass_utils import run_bass_kernel_spmd

BF16 = mybir.dt.bfloat16
F32 = mybir.dt.float32
MUL = mybir.AluOpType.mult
ADD = mybir.AluOpType.add

B, H, W = 8, 352, 1216
HP, WPAD = H + 2, W + 2        # 354, 1218
GROWS = 1 + 9 * HP + 13        # 3200
SROWS = 368
N_CORES = 8
CHUNKS = [(0, 126, 128), (126, 126, 128), (252, 100, 112)]  # (y0, R, KL)
STRIPS = [(0, 512), (512, 512), (1024, 192)]


def make_shift_mats():
    """S_d[k, m] = 1 iff k == m + d, d in {0,1,2}; packed [128, 378] bf16."""
    sm = np.zeros((128, 3 * 126), ml_dtypes.bfloat16)
    for d in range(3):
        for m in range(126):
            sm[m + d, d * 126 + m] = 1.0
    return sm


def prep_core_inputs(guide_b: np.ndarray, hn_b: np.ndarray, h0_b: np.ndarray,
                     sm: np.ndarray) -> dict:
    """guide_b [9, 354, 1218] f32, hn_b/h0_b [352, 1216] f32 -> bf16 dram dict."""
    gp = np.zeros((GROWS, W), ml_dtypes.bfloat16)
    gp[1:1 + 9 * HP] = np.asarray(guide_b, np.float32)[:, :, 1:1 + W].reshape(9 * HP, W)
    hnp = np.zeros((SROWS, WPAD), ml_dtypes.bfloat16)
    hnp[1:1 + H, 1:1 + W] = hn_b
    h0p = np.zeros((SROWS, W), ml_dtypes.bfloat16)
    h0p[1:1 + H, :] = h0_b
    return {"guide": gp, "hn": hnp, "h0": h0p, "smat": sm}


def build():
    nc = bacc.Bacc(enable_partition_id=False)
    g_d = nc.dram_tensor("guide", [GROWS, W], BF16, kind="ExternalInput")
    hn_d = nc.dram_tensor("hn", [SROWS, WPAD], BF16, kind="ExternalInput")
    h0_d = nc.dram_tensor("h0", [SROWS, W], BF16, kind="ExternalInput")
    sm_d = nc.dram_tensor("smat", [128, 3 * 126], BF16, kind="ExternalInput")
    out_d = nc.dram_tensor("out", [H, W], BF16, kind="ExternalOutput")

    with tile.TileContext(nc) as tc:
        with tc.tile_pool(name="const", bufs=1) as cpool, \
             tc.tile_pool(name="gpool", bufs=3) as gpool, \
             tc.tile_pool(name="spool", bufs=3) as spool, \
             tc.tile_pool(name="ppool", bufs=2) as ppool, \
             tc.tile_pool(name="opool", bufs=2) as opool, \
             tc.tile_pool(name="psum", bufs=2, space="PSUM") as pspool:

            smt = cpool.tile([128, 3 * 126], BF16)
            nc.sync.dma_start(out=smt[0:64, :], in_=sm_d[0:64, :])
            nc.scalar.dma_start(out=smt[64:128, :], in_=sm_d[64:128, :])

            st = {}   # per-chunk tiles

            def issue_smalls(ci):
                y0, R, KL = CHUNKS[ci]
                hnt = spool.tile([128, WPAD], BF16, tag="hn", name="hnt")
                h0t = spool.tile([128, W], BF16, tag="h0", name="h0t")
                if KL == 128:
                    nc.sync.dma_start(out=hnt[0:64, :], in_=hn_d[y0:y0 + 64, :])
                    nc.scalar.dma_start(out=hnt[64:128, :],
                                        in_=hn_d[y0 + 64:y0 + 128, :])
                    nc.scalar.dma_start(out=h0t[0:64, :], in_=h0_d[y0:y0 + 64, :])
                    nc.sync.dma_start(out=h0t[64:128, :],
                                      in_=h0_d[y0 + 64:y0 + 128, :])
                else:
                    nc.sync.dma_start(out=hnt[0:KL, :], in_=hn_d[y0:y0 + KL, :])
                    nc.gpsimd.dma_start(out=h0t[0:KL, :], in_=h0_d[y0:y0 + KL, :])
                st[ci] = {"hnt": hnt, "h0t": h0t}

            rings = [nc.sync, nc.scalar, nc.gpsimd]

            def issue_band(ci, a):
                """Guide band a: planes 3a..3a+2, tile row k <- flat row
                1 + (3a+p)*HP + y0 + a - 1 + k."""
                y0, R, KL = CHUNKS[ci]
                if a == 0:
                    st[ci]["gt"] = gpool.tile([128, 9, W], BF16, tag="g", name="gt")
                gt = st[ci]["gt"]
                base = 1 + 3 * a * HP + y0 + a - 1
                rings[a].dma_start(
                    out=gt[0:KL, 3 * a:3 * a + 3, :],
                    in_=AP(g_d, base * W, [[W, KL], [HP * W, 3], [1, W]]))

            def compute(ci):
                y0, R, KL = CHUNKS[ci]
                hnt, h0t, gt = st[ci]["hnt"], st[ci]["h0t"], st[ci]["gt"]

                def src_for(t):
                    i, j = t // 3, t % 3
                    if t == 4:
                        return h0t[0:KL, :]
                    # j=1 taps read hn at odd offset 1 (1x DVE, still correct)
                    return hnt[0:KL, 2 - j:2 - j + W]

                pt = ppool.tile([128, 9, W], BF16, tag="p", name="pt")
                q = ppool.tile([128, 2, W], BF16, tag="q", name="qt")
                qb = ppool.tile([128, W], BF16, tag="qb", name="qbt")
                for t in (0, 1, 2, 3, 4, 5):
                    nc.vector.tensor_tensor(pt[0:KL, t], gt[0:KL, t], src_for(t), MUL)
                nc.vector.tensor_tensor(
                    q[0:KL],
                    AP(pt.tensor, 0, [[9 * W, KL], [3 * W, 2], [1, W]]),
                    AP(pt.tensor, W, [[9 * W, KL], [3 * W, 2], [1, W]]),
                    ADD)
                for t in (6, 7):
                    nc.vector.tensor_tensor(pt[0:KL, t], gt[0:KL, t], src_for(t), MUL)
                nc.vector.tensor_tensor(qb[0:KL], pt[0:KL, 6], pt[0:KL, 7], ADD)
                nc.vector.tensor_tensor(pt[0:KL, 8], gt[0:KL, 8], src_for(8), MUL)

                psts = [pspool.tile([126, 512], F32, tag=f"ps{s}", name=f"ps{s}")
                        for s in range(len(STRIPS))]
                streams = [(q[0:KL, 0, :], 2), (q[0:KL, 1, :], 1),
                           (pt[0:KL, 2, :], 2), (pt[0:KL, 5, :], 1),
                           (qb[0:KL, :], 0), (pt[0:KL, 8, :], 0)]
                for mi, (mv, d) in enumerate(streams):
                    for s, (w0, N) in enumerate(STRIPS):
                        nc.tensor.matmul(psts[s][0:R, 0:N],
                                         smt[0:KL, d * 126:d * 126 + R],
                                         mv[:, w0:w0 + N],
                                         start=(mi == 0), stop=(mi == 5))

                ot = opool.tile([128, W], BF16, tag="out", name="ot")
                for s, (w0, N) in enumerate(STRIPS):
                    nc.scalar.copy(out=ot[0:R, w0:w0 + N], in_=psts[s][0:R, 0:N])
                st[ci]["ot"] = ot

            def store(ci, ring=None):
                y0, R, KL = CHUNKS[ci]
                ot = st[ci]["ot"]
                if ring is not None:
                    ring.dma_start(out=out_d[y0:y0 + R, :], in_=ot[0:R, :])
                else:
                    for s, (w0, N) in enumerate(STRIPS):
                        rings[(s + 2) % 3].dma_start(
                            out=out_d[y0:y0 + R, w0:w0 + N],
                            in_=ot[0:R, w0:w0 + N])

            # global just-in-time issue order; round-robin rings per band
            issue_smalls(0)
            issue_band(0, 0)
            issue_band(0, 1)
            issue_smalls(1)
            issue_band(0, 2)
            issue_band(1, 0)
            issue_band(1, 1)
            issue_smalls(2)
            compute(0)
            issue_band(1, 2)
            issue_band(2, 0)
            issue_band(2, 1)
            store(0, ring=nc.gpsimd)
            compute(1)
            issue_band(2, 2)
            store(1, ring=nc.gpsimd)
            compute(2)
            store(2)

    nc.finalize()
    return nc


_nc_cache = {}


def _get_nc():
    if "nc" not in _nc_cache:
        _nc_cache["nc"] = build()
    return _nc_cache["nc"]


def kernel(guide_weight: np.ndarray, hn: np.ndarray, h0: np.ndarray) -> np.ndarray:
    """Full inputs: guide_weight [8,9,354,1218], hn/h0 [8,1,352,1216] f32.
    Returns [8,1,352,1216] f32."""
    nc = _get_nc()
    sm = make_shift_mats()
    in_maps = [prep_core_inputs(guide_weight[b], hn[b, 0], h0[b, 0], sm)
               for b in range(B)]
    res = run_bass_kernel_spmd(nc, in_maps, list(range(N_CORES)))
    out = np.stack([np.asarray(res.results[b]["out"]) for b in range(B)], axis=0)
    return out[:, None].astype(np.float32)
